# revision 1
# baseline (speedup 1.0000x reference)
"""ClusterLoss Bass/Tile kernel for Trainium2 (8 NeuronCores, data parallel).

Strategy
--------
Pure data parallelism over B=2048 with a count-aware schedule: samples are
globally sorted by hn_count and dealt into 16 blocks of 128; every core gets
one "big" block (slot 0, padded negative bound N0) and one "small" block
(slot 1, bound N1), so a single SPMD program with static loop bounds fits all
cores while skipping most padded negatives.  The four losses are sums over
samples, so no output unpermutation is needed.

Per block, samples sit on the 128 partitions with the feature dim d on the
free axis.  The einsum('bd,bnd->bn') runs as one fused DVE tensor_tensor
multiply per 8-negative chunk (q broadcast via a stride-0 access pattern)
with the per-negative free-axis reductions on the otherwise idle Scalar
engine (activation Copy with accum_out, folding the 1/TEMP scale).  Masked
logsumexp / cross-entropy / BML terms are small per-partition vector ops.
Each core emits 5 partial sums reduced over partitions with a ones-vector
matmul; the final scalar combine runs on host.

The program is JIT-specialized to (N0, N1) derived from the counts at call
time and cached, so repeated calls with the same raggedness profile reuse
the compiled NEFF.
"""

from contextlib import ExitStack

import numpy as np

import concourse.bass as bass
import concourse.bacc as bacc
import concourse.tile as tile
from concourse import mybir
from concourse import bass_utils

N_CORES = 8
B, D, N_MAX, M_MAX = 2048, 512, 256, 32
B_LOC = B // N_CORES          # 256 samples per core
PBLK = 128                    # partition block
NBLK = B_LOC // PBLK          # 2 slots per core
NCHUNK = 16                   # negatives per hn DMA tile / DVE multiply

TEMP, ALPHA, BETA, LAMBDA_BML = 0.07, 0.4, 0.2, 0.2
NEG = -1e30
EXP_CLAMP = -87.0             # exp(-87) underflows f32; avoids LUT extremes

F32 = mybir.dt.float32
F16 = mybir.dt.float16
BF16 = mybir.dt.bfloat16
I32 = mybir.dt.int32
AF = mybir.ActivationFunctionType
OP = mybir.AluOpType
AX = mybir.AxisListType


def _bcast_n(ap, n):
    """(128, D) AP viewed as (128, n, D) with stride-0 broadcast on n."""
    return bass.AP(tensor=ap.tensor, offset=ap.offset,
                   ap=[ap.ap[0], [0, n], ap.ap[1]])


def _emit(tc, bounds, q, q16, k, k2, hns, fn, hc, fc, out):
    nc = tc.nc
    with ExitStack() as ctx:
        hpool = ctx.enter_context(tc.tile_pool(name="hnp", bufs=5))
        fpool = ctx.enter_context(tc.tile_pool(name="fnp", bufs=2))
        qpool = ctx.enter_context(tc.tile_pool(name="qkp", bufs=2))
        mpool = ctx.enter_context(tc.tile_pool(name="med", bufs=2))
        spool = ctx.enter_context(tc.tile_pool(name="scr", bufs=2))
        smpool = ctx.enter_context(tc.tile_pool(name="sm", bufs=2))
        cpool = ctx.enter_context(tc.tile_pool(name="cst", bufs=1))
        ppool = ctx.enter_context(tc.tile_pool(name="ps", bufs=2, space="PSUM"))

        # constants
        iota_i = cpool.tile([PBLK, N_MAX], I32, tag="iota_i", name="iota_i")
        nc.gpsimd.iota(iota_i[:], pattern=[[1, N_MAX]], base=0, channel_multiplier=0)
        iota_f = cpool.tile([PBLK, N_MAX], F32, tag="iota_f", name="iota_f")
        nc.vector.tensor_copy(out=iota_f[:], in_=iota_i[:])
        ones = cpool.tile([PBLK, 1], F32, tag="ones", name="ones")
        nc.vector.memset(ones[:], 1.0)
        alpha_t = cpool.tile([PBLK, 1], F32, tag="alpha_t", name="alpha_t")
        nc.vector.memset(alpha_t[:], ALPHA)
        nbeta_t = cpool.tile([PBLK, 1], F32, tag="nbeta_t", name="nbeta_t")
        nc.vector.memset(nbeta_t[:], -BETA)

        def sm(tag, dt=F32, w=1):
            return smpool.tile([PBLK, w], dt, tag=tag, name=tag)

        blk_contribs = []
        for s in range(NBLK):
            NS = bounds[s]
            q_t = qpool.tile([PBLK, D], F32, tag="q_t", name="q_t")
            nc.sync.dma_start(out=q_t[:], in_=q[s])
            k_t = qpool.tile([PBLK, D], F32, tag="k_t", name="k_t")
            nc.sync.dma_start(out=k_t[:], in_=k[s])
            k2_t = qpool.tile([PBLK, D], F32, tag="k2_t", name="k2_t")
            nc.sync.dma_start(out=k2_t[:], in_=k2[s])
            q16_t = qpool.tile([PBLK, D], BF16, tag="q16_t", name="q16_t")
            nc.sync.dma_start(out=q16_t[:], in_=q16[s])
            hc_i = sm("hc_i", I32)
            nc.sync.dma_start(out=hc_i[:], in_=hc[s])
            fc_i = sm("fc_i", I32)
            nc.sync.dma_start(out=fc_i[:], in_=fc[s])
            hc_f = sm("hc_f")
            nc.vector.tensor_copy(out=hc_f[:], in_=hc_i[:])
            fc_f = sm("fc_f")
            nc.vector.tensor_copy(out=fc_f[:], in_=fc_i[:])

            dots = spool.tile([PBLK, D], F32, tag="dots", name="dots")
            adump = spool.tile([PBLK, D], F16, tag="adump", name="adump", bufs=1)

            def rowdot(in1, scale, accum):
                nc.vector.scalar_tensor_tensor(
                    out=dots[:], in0=q_t[:], scalar=scale, in1=in1,
                    op0=OP.mult, op1=OP.mult, accum_out=accum,
                )

            lpos = sm("lpos")
            rowdot(k_t[:], 1.0 / TEMP, lpos[:])
            lposnb = sm("lposnb")
            rowdot(k2_t[:], 1.0 / TEMP, lposnb[:])
            simpos = sm("simpos")
            rowdot(k_t[:], 1.0, simpos[:])

            # negative logits: lneg[b, n] = q.hn[b, n] / TEMP
            # DVE does one (128, 8, 512) multiply per chunk (q broadcast on
            # n, 1/TEMP folded in); per-negative free-axis reductions are
            # split R_DVE:NCHUNK-R_DVE between DVE (one multi-n
            # tensor_reduce) and the otherwise idle ScalarE (Copy+accum) so
            # both engines run near their line rate.
            lneg = mpool.tile([PBLK, N_MAX], F32, tag="lneg", name="lneg")
            for c in range(NS // NCHUNK):
                r_dve = 8 if c % 2 else 7
                n0 = c * NCHUNK
                h_t = hpool.tile([PBLK, NCHUNK, D], BF16, tag="h_t", name="h_t")
                nc.sync.dma_start(
                    out=h_t[:], in_=hns[s][:, n0:n0 + NCHUNK, :]
                )
                prod = spool.tile([PBLK, NCHUNK, D], F16, tag="prod", name="prod")
                nc.vector.tensor_mul(out=prod[:], in0=h_t[:],
                                     in1=_bcast_n(q16_t[:], NCHUNK))
                nc.vector.tensor_reduce(
                    out=lneg[:, n0:n0 + r_dve], in_=prod[:, :r_dve, :],
                    axis=AX.X, op=OP.add,
                )
                for j in range(r_dve, NCHUNK):
                    nc.scalar.activation(
                        out=adump[:], in_=prod[:, j, :], func=AF.Copy,
                        scale=1.0, accum_out=lneg[:, n0 + j:n0 + j + 1],
                    )
            # bf16 tensor_tensor cannot fold a scale; apply 1/TEMP once here
            nc.vector.tensor_scalar_mul(out=lneg[:, :NS], in0=lneg[:, :NS],
                                        scalar1=1.0 / TEMP)

            # fn dots: q.fn[b, m]; same split, no TEMP scale
            fnd = sm("fnd", w=M_MAX)
            for c in range(M_MAX // NCHUNK):
                r_dve = 8 if c % 2 else 7
                m0 = c * NCHUNK
                f_t = fpool.tile([PBLK, NCHUNK, D], BF16, tag="f_t", name="f_t")
                nc.sync.dma_start(
                    out=f_t[:], in_=fn[s, :, m0:m0 + NCHUNK, :]
                )
                prodf = spool.tile([PBLK, NCHUNK, D], F16, tag="prod", name="prodf")
                nc.vector.tensor_mul(out=prodf[:], in0=f_t[:],
                                     in1=_bcast_n(q16_t[:], NCHUNK))
                nc.vector.tensor_reduce(
                    out=fnd[:, m0:m0 + r_dve], in_=prodf[:, :r_dve, :],
                    axis=AX.X, op=OP.add,
                )
                for j in range(r_dve, NCHUNK):
                    nc.scalar.activation(
                        out=adump[:], in_=prodf[:, j, :], func=AF.Copy,
                        scale=1.0, accum_out=fnd[:, m0 + j:m0 + j + 1],
                    )

            # mask padded negatives to -1e30, then logsumexp along free axis
            mneg = mpool.tile([PBLK, N_MAX], F32, tag="mneg", name="mneg")
            nc.vector.tensor_scalar(
                out=mneg[:, :NS], in0=iota_f[:, :NS], scalar1=hc_f[:],
                scalar2=NEG, op0=OP.is_ge, op1=OP.mult,
            )
            nc.vector.tensor_add(out=lneg[:, :NS], in0=lneg[:, :NS],
                                 in1=mneg[:, :NS])
            mrow = sm("mrow")
            nc.vector.tensor_reduce(out=mrow[:], in_=lneg[:, :NS], axis=AX.X,
                                    op=OP.max)
            nmrow = sm("nmrow")
            nc.vector.tensor_scalar_mul(out=nmrow[:], in0=mrow[:], scalar1=-1.0)
            expin = mpool.tile([PBLK, N_MAX], F32, tag="expin", name="expin")
            nc.vector.tensor_scalar(
                out=expin[:, :NS], in0=lneg[:, :NS], scalar1=nmrow[:],
                scalar2=EXP_CLAMP, op0=OP.add, op1=OP.max,
            )
            expout = mpool.tile([PBLK, N_MAX], F32, tag="expout", name="expout")
            sumexp = sm("sumexp")
            nc.scalar.activation(
                out=expout[:, :NS], in_=expin[:, :NS], func=AF.Exp,
                accum_out=sumexp[:],
            )
            lse = sm("lse")
            nc.scalar.activation(out=lse[:], in_=sumexp[:], func=AF.Ln)
            nc.vector.tensor_add(out=lse[:], in0=lse[:], in1=mrow[:])

            # ce(lp) = logaddexp(lp, lse) - lp
            def ce(lp, tag):
                mm = sm("mm" + tag)
                nc.vector.tensor_max(out=mm[:], in0=lp[:], in1=lse[:])
                nmm = sm("nmm" + tag)
                nc.vector.tensor_scalar_mul(out=nmm[:], in0=mm[:], scalar1=-1.0)
                e1 = sm("e1" + tag)
                nc.vector.tensor_scalar(
                    out=e1[:], in0=lp[:], scalar1=nmm[:], scalar2=EXP_CLAMP,
                    op0=OP.add, op1=OP.max,
                )
                nc.scalar.activation(out=e1[:], in_=e1[:], func=AF.Exp)
                e2 = sm("e2" + tag)
                nc.vector.tensor_scalar(
                    out=e2[:], in0=lse[:], scalar1=nmm[:], scalar2=EXP_CLAMP,
                    op0=OP.add, op1=OP.max,
                )
                nc.scalar.activation(out=e2[:], in_=e2[:], func=AF.Exp)
                s12 = sm("s12" + tag)
                nc.vector.tensor_add(out=s12[:], in0=e1[:], in1=e2[:])
                nc.scalar.activation(out=s12[:], in_=s12[:], func=AF.Ln)
                cev = sm("ce" + tag)
                nc.vector.tensor_add(out=cev[:], in0=s12[:], in1=mm[:])
                nc.vector.tensor_sub(out=cev[:], in0=cev[:], in1=lp[:])
                return cev

            cep = ce(lpos, "p")
            cenb = ce(lposnb, "n")

            # BML term
            maskf = sm("maskf", w=M_MAX)
            nc.vector.tensor_scalar(
                out=maskf[:], in0=iota_f[:, :M_MAX], scalar1=fc_f[:],
                scalar2=None, op0=OP.is_lt,
            )
            nc.vector.tensor_mul(out=fnd[:], in0=fnd[:], in1=maskf[:])
            sfn = sm("sfn")
            nc.vector.tensor_reduce(out=sfn[:], in_=fnd[:], axis=AX.X, op=OP.add)
            den = sm("den")
            nc.vector.tensor_scalar_max(out=den[:], in0=fc_f[:], scalar1=1.0)
            rden = sm("rden")
            nc.vector.reciprocal(out=rden[:], in_=den[:])
            simfn = sm("simfn")
            nc.vector.tensor_mul(out=simfn[:], in0=sfn[:], in1=rden[:])
            delta = sm("delta")
            nc.vector.tensor_sub(out=delta[:], in0=simfn[:], in1=simpos[:])
            r1 = sm("r1")
            nc.scalar.activation(out=r1[:], in_=delta[:], func=AF.Relu,
                                 bias=alpha_t[:], scale=1.0)
            r2 = sm("r2")
            nc.scalar.activation(out=r2[:], in_=delta[:], func=AF.Relu,
                                 bias=nbeta_t[:], scale=-1.0)
            bml = sm("bml")
            nc.vector.tensor_add(out=bml[:], in0=r1[:], in1=r2[:])

            vh = sm("vh")
            nc.vector.tensor_scalar(out=vh[:], in0=hc_f[:], scalar1=0.0,
                                    scalar2=None, op0=OP.is_gt)
            vf = sm("vf")
            nc.vector.tensor_scalar(out=vf[:], in0=fc_f[:], scalar1=0.0,
                                    scalar2=None, op0=OP.is_gt)
            vb = sm("vb")
            nc.vector.tensor_mul(out=vb[:], in0=vh[:], in1=vf[:])

            contrib = smpool.tile([PBLK, 5], F32, tag="contrib", name="contrib")
            nc.vector.tensor_mul(out=contrib[:, 0:1], in0=cep[:], in1=vh[:])
            nc.vector.tensor_mul(out=contrib[:, 1:2], in0=cenb[:], in1=vh[:])
            nc.vector.tensor_mul(out=contrib[:, 2:3], in0=bml[:], in1=vb[:])
            nc.vector.tensor_copy(out=contrib[:, 3:4], in_=vh[:])
            nc.vector.tensor_copy(out=contrib[:, 4:5], in_=vb[:])
            blk_contribs.append(contrib)

        tot = blk_contribs[0]
        nc.vector.tensor_add(out=tot[:], in0=tot[:], in1=blk_contribs[1][:])

        ps = ppool.tile([5, 1], F32, tag="ps5", name="ps5")
        nc.tensor.matmul(ps[:], lhsT=tot[:], rhs=ones[:], start=True, stop=True)
        res = smpool.tile([5, 1], F32, tag="res", name="res")
        nc.scalar.copy(out=res[:], in_=ps[:])
        nc.sync.dma_start(out=out[:], in_=res[:])


def _build(bounds):
    N0, N1 = bounds
    nc = bacc.Bacc("TRN2", target_bir_lowering=False, debug=False)
    q = nc.dram_tensor("q", [NBLK, PBLK, D], F32, kind="ExternalInput")
    k = nc.dram_tensor("k", [NBLK, PBLK, D], F32, kind="ExternalInput")
    k2 = nc.dram_tensor("k2", [NBLK, PBLK, D], F32, kind="ExternalInput")
    q16 = nc.dram_tensor("q16", [NBLK, PBLK, D], BF16, kind="ExternalInput")
    hn0 = nc.dram_tensor("hn0", [PBLK, N0, D], BF16, kind="ExternalInput")
    hn1 = nc.dram_tensor("hn1", [PBLK, N1, D], BF16, kind="ExternalInput")
    fn = nc.dram_tensor("fn", [NBLK, PBLK, M_MAX, D], BF16, kind="ExternalInput")
    hc = nc.dram_tensor("hn_counts", [NBLK, PBLK, 1], I32, kind="ExternalInput")
    fc = nc.dram_tensor("fn_counts", [NBLK, PBLK, 1], I32, kind="ExternalInput")
    out = nc.dram_tensor("out", [5, 1], F32, kind="ExternalOutput")
    with tile.TileContext(nc) as tc:
        _emit(tc, bounds, q, q16, k, k2, (hn0, hn1), fn, hc, fc, out)
    nc.compile()
    return nc


_NC_CACHE = {}


def _get_nc(bounds):
    if bounds not in _NC_CACHE:
        _NC_CACHE[bounds] = _build(bounds)
    return _NC_CACHE[bounds]


def _round8(x):
    return max(NCHUNK, int(-(-int(x) // NCHUNK) * NCHUNK))


def plan(hn_counts):
    """Global count-sorted block schedule: returns (order, (N0, N1))."""
    order = np.argsort(-hn_counts, kind="stable")
    blocks = order.reshape(2 * N_CORES, PBLK)
    c = np.asarray(hn_counts)
    n0 = _round8(c[blocks[0:N_CORES]].max())
    n1 = _round8(c[blocks[N_CORES:]].max())
    return blocks, (min(n0, N_MAX), min(n1, N_MAX))


def make_in_maps(q, k, k2, hn, fn, hn_counts, fn_counts):
    q = np.asarray(q, np.float32)
    k = np.asarray(k, np.float32)
    k2 = np.asarray(k2, np.float32)
    hn = np.asarray(hn, np.float32)
    fn = np.asarray(fn, np.float32)
    hn_counts = np.asarray(hn_counts, np.int32)
    fn_counts = np.asarray(fn_counts, np.int32)
    blocks, (n0, n1) = plan(hn_counts)
    import ml_dtypes
    q16 = q.astype(ml_dtypes.bfloat16)
    hn16 = hn.astype(ml_dtypes.bfloat16)
    fn16 = fn.astype(ml_dtypes.bfloat16)
    hn_v0 = hn16[:, :n0, :]   # views, no copy
    hn_v1 = hn16[:, :n1, :]
    in_maps = []
    for c in range(N_CORES):
        i0, i1 = blocks[c], blocks[N_CORES + c]
        both = np.stack([i0, i1])
        in_maps.append({
            "q": q[both],
            "q16": q16[both],
            "k": k[both],
            "k2": k2[both],
            "hn0": hn_v0[i0],
            "hn1": hn_v1[i1],
            "fn": fn16[both],
            "hn_counts": hn_counts[both][..., None],
            "fn_counts": fn_counts[both][..., None],
        })
    return in_maps, (n0, n1)


def combine_partials(results):
    parts = np.stack([np.asarray(r["out"], np.float64).reshape(5) for r in results])
    cl_s, clnb_s, bml_s, nv, nb = parts.sum(axis=0)
    n_valid = max(nv, 1.0)
    cl = cl_s / n_valid
    clnb = clnb_s / n_valid
    bml_mean = (bml_s / nb) if nb > 0 else 0.0
    lbml = LAMBDA_BML * bml_mean
    tot = cl + clnb + lbml
    return np.array([tot, cl, lbml, clnb], np.float32)


def run_spmd(in_maps, bounds, **kwargs):
    nc = _get_nc(bounds)
    return bass_utils.run_bass_kernel_spmd(
        nc, in_maps, core_ids=list(range(N_CORES)), **kwargs
    )


def kernel(q, k, k2, hn, fn, hn_counts, fn_counts):
    in_maps, bounds = make_in_maps(q, k, k2, hn, fn, hn_counts, fn_counts)
    res = run_spmd(in_maps, bounds)
    return combine_partials(res.results)



# revision 6
# speedup vs baseline: 1.6159x; 1.6159x over previous
"""ClusterLoss Bass/Tile kernel for Trainium2 (8 NeuronCores, data parallel).

Strategy (v2, TensorE-centric)
------------------------------
The dominant work is einsum('bd,bnd->bn') over hn (1GB). Instead of
DVE multiply+reduce (engine-bound at ~0.96GHz), the einsum runs on the
TensorEngine as block-diagonal matmuls over host-transposed fp8 data:

- Samples are globally sorted by hn_count and dealt to 8 cores x 2
  slots x 128 lanes (sorted within each slot). Each slot splits into 4
  blocks of G=32 lanes; per-block negative bounds B[s][b] are the max
  count in the block (rank-256 granularity), rounded to 16. This cuts
  the padded-negative footprint from n_avg=192 (2-bound schedule) to
  ~144.
- Host packs hn/fn/q transposed (d on partitions) in fp8-e4m3, exactly
  in the per-round layout the kernel streams: round r of a slot loads
  16 negatives x 32 samples for each active block (<=4 x 512 columns).
- Per round, 4 K-tile matmuls per active block (lhsT = block's qT,
  rhs = hnT columns) produce the all-pairs tile in PSUM at partition
  base 32*b; ScalarE copies PSUM->SBUF (bf16), a DMA dumps it to a
  DRAM scratch, and a strided DMA gathers the diagonal (the true
  per-sample logits) back into per-slot lneg strips. (SBUF APs cannot
  express per-partition offsets, DRAM APs can -- hence the bounce.)
- Downstream masked-logsumexp / cross-entropy / BML terms are small
  per-partition DVE/ScalarE ops identical in structure to the v1
  kernel; 1/TEMP is folded into the mask-add pass. Per-core partial
  sums reduce over partitions with a ones-vector matmul; the final
  scalar combine runs on host.

The program is JIT-specialized to the 8 block bounds derived from the
counts at call time and cached, so repeated calls with the same
raggedness profile reuse the compiled NEFF.
"""

from contextlib import ExitStack

import numpy as np

import concourse.bass as bass
import concourse.bacc as bacc
import concourse.tile as tile
from concourse import mybir
from concourse import bass_utils

N_CORES = 8
B, D, N_MAX, M_MAX = 2048, 512, 256, 32
PBLK = 128                    # lanes per slot
NSLOT = 2                     # slots per core
G = 32                        # samples per block
NB = PBLK // G                # blocks per slot
JC = 16                       # negatives per round
KT = 4                        # 128-row K-tiles in D
COLS = G * JC                 # matmul moving columns per block-round

TEMP, ALPHA, BETA, LAMBDA_BML = 0.07, 0.4, 0.2, 0.2
NEG = -1e30
EXP_CLAMP = -87.0

F32 = mybir.dt.float32
BF16 = mybir.dt.bfloat16
F8 = mybir.dt.float8e4
I32 = mybir.dt.int32
AF = mybir.ActivationFunctionType
OP = mybir.AluOpType
AX = mybir.AxisListType


def _rounds(bounds_s):
    """Per-round active-block counts for one slot's hn stream."""
    nr = bounds_s[0] // JC
    return [sum(1 for b in bounds_s if b > JC * r) for r in range(nr)]


def _emit(tc, bounds, ht, qt, scr, q, k, k2, hc, fc, out):
    nc = tc.nc
    SCR_W = scr.shape[1]
    with ExitStack() as ctx:
        hpool = ctx.enter_context(tc.tile_pool(name="ht", bufs=3))
        stpool = ctx.enter_context(tc.tile_pool(name="stg", bufs=3))
        qpool = ctx.enter_context(tc.tile_pool(name="qk", bufs=1))
        lpool = ctx.enter_context(tc.tile_pool(name="lneg", bufs=1))
        smpool = ctx.enter_context(tc.tile_pool(name="sm", bufs=1))
        cpool = ctx.enter_context(tc.tile_pool(name="cst", bufs=1))
        ppool = ctx.enter_context(tc.tile_pool(name="ps", bufs=4, space="PSUM"))
        rpool = ctx.enter_context(tc.tile_pool(name="rs", bufs=1, space="PSUM"))

        # constants
        iota_i = cpool.tile([PBLK, N_MAX], I32, tag="iota_i", name="iota_i")
        nc.gpsimd.iota(iota_i[:], pattern=[[1, N_MAX]], base=0, channel_multiplier=0)
        iota_f = cpool.tile([PBLK, N_MAX], BF16, tag="iota_f", name="iota_f")
        nc.vector.tensor_copy(out=iota_f[:], in_=iota_i[:])
        ones = cpool.tile([PBLK, 1], F32, tag="ones", name="ones")
        nc.vector.memset(ones[:], 1.0)
        alpha_t = cpool.tile([PBLK, 1], F32, tag="alpha_t", name="alpha_t")
        nc.vector.memset(alpha_t[:], ALPHA)
        nbeta_t = cpool.tile([PBLK, 1], F32, tag="nbeta_t", name="nbeta_t")
        nc.vector.memset(nbeta_t[:], -BETA)

        # per-slot q (transposed fp8) for the PE streams
        qt_t = []
        for s in range(NSLOT):
            t = qpool.tile([PBLK, KT, PBLK], F8, tag=f"qt{s}", name=f"qt{s}")
            nc.sync.dma_start(out=t[:], in_=qt[s])
            qt_t.append(t)

        lneg = []
        fnd = []
        for s in range(NSLOT):
            lneg.append(lpool.tile([PBLK, bounds[s][0]], BF16,
                                   tag=f"lneg{s}", name=f"lneg{s}"))
            fnd.append(lpool.tile([PBLK, M_MAX], BF16,
                                  tag=f"fnd{s}", name=f"fnd{s}"))

        # ---- PE einsum streams ----
        ht_off = 0
        scr_off = 0

        def stream_round(s, dest, dw, dcol, nact):
            """One round: load, matmul per active block, copy, dump, gather."""
            nonlocal ht_off, scr_off
            h = hpool.tile([PBLK, KT, NB * COLS], F8, tag="h", name=f"h{scr_off}")
            nc.sync.dma_start(out=h[:, :, 0:nact * COLS],
                              in_=ht[:, ht_off:ht_off + KT * nact * COLS])
            ps = ppool.tile([PBLK, COLS], F32, tag="ps", name=f"ps{scr_off}")
            for b in range(nact):
                for kt in range(KT):
                    nc.tensor.matmul(
                        ps[G * b:G * (b + 1), :],
                        lhsT=qt_t[s][:, kt, G * b:G * (b + 1)],
                        rhs=h[:, kt, COLS * b:COLS * (b + 1)],
                        start=(kt == 0), stop=(kt == KT - 1),
                        tile_position=(0, G * b),
                    )
            stg = stpool.tile([PBLK, COLS], BF16, tag="stg", name=f"stg{scr_off}")
            nc.scalar.copy(out=stg[0:G * nact, :], in_=ps[0:G * nact, :])
            nc.sync.dma_start(out=scr[0:G * nact, scr_off:scr_off + COLS],
                              in_=stg[0:G * nact, :])
            src = bass.AP(
                tensor=scr[:].tensor, offset=scr[:].offset + scr_off,
                ap=[[G * SCR_W, nact], [SCR_W + JC, G], [1, JC]],
            )
            dst = bass.AP(
                tensor=dest[:].tensor, offset=dest[:].offset + dcol,
                ap=[[dw, G * nact], [1, JC]],
            )
            nc.sync.dma_start(out=dst, in_=src)
            ht_off += KT * nact * COLS
            scr_off += COLS

        for s in range(NSLOT):
            for r, nact in enumerate(_rounds(bounds[s])):
                stream_round(s, lneg[s], bounds[s][0], JC * r, nact)
            for r in range(M_MAX // JC):
                stream_round(s, fnd[s], M_MAX, JC * r, NB)

        # ---- downstream per-slot scalar math ----
        def sm(tag, dt=F32, w=1):
            return smpool.tile([PBLK, w], dt, tag=tag, name=tag)

        blk_contribs = []
        for s in range(NSLOT):
            NS = bounds[s][0]
            q_t = qpool.tile([PBLK, D], F32, tag=f"q_t{s}", name=f"q_t{s}")
            nc.sync.dma_start(out=q_t[:], in_=q[s])
            k_t = qpool.tile([PBLK, D], F32, tag=f"k_t{s}", name=f"k_t{s}")
            nc.sync.dma_start(out=k_t[:], in_=k[s])
            k2_t = qpool.tile([PBLK, D], F32, tag=f"k2_t{s}", name=f"k2_t{s}")
            nc.sync.dma_start(out=k2_t[:], in_=k2[s])
            hc_i = sm(f"hc_i{s}", I32)
            nc.sync.dma_start(out=hc_i[:], in_=hc[s])
            fc_i = sm(f"fc_i{s}", I32)
            nc.sync.dma_start(out=fc_i[:], in_=fc[s])
            hc_f = sm(f"hc_f{s}")
            nc.vector.tensor_copy(out=hc_f[:], in_=hc_i[:])
            fc_f = sm(f"fc_f{s}")
            nc.vector.tensor_copy(out=fc_f[:], in_=fc_i[:])

            def rowdot(in1, scale, accum):
                dots = smpool.tile([PBLK, D], F32, tag="dots", name="dots")
                nc.vector.scalar_tensor_tensor(
                    out=dots[:], in0=q_t[:], scalar=scale, in1=in1,
                    op0=OP.mult, op1=OP.mult, accum_out=accum,
                )

            lpos = sm(f"lpos{s}")
            rowdot(k_t[:], 1.0 / TEMP, lpos[:])
            lposnb = sm(f"lposnb{s}")
            rowdot(k2_t[:], 1.0 / TEMP, lposnb[:])
            simpos = sm(f"simpos{s}")
            rowdot(k_t[:], 1.0, simpos[:])

            # mask padded negatives to NEG and fold in 1/TEMP
            mneg = lpool.tile([PBLK, NS], BF16, tag=f"mneg{s}", name=f"mneg{s}")
            nc.vector.tensor_scalar(
                out=mneg[:], in0=iota_f[:, :NS], scalar1=hc_f[:],
                scalar2=NEG, op0=OP.is_ge, op1=OP.mult,
            )
            lm = lpool.tile([PBLK, NS], BF16, tag=f"lm{s}", name=f"lm{s}")
            nc.vector.scalar_tensor_tensor(
                out=lm[:], in0=lneg[s][:], scalar=1.0 / TEMP, in1=mneg[:],
                op0=OP.mult, op1=OP.add,
            )
            mrow = sm(f"mrow{s}")
            nc.vector.tensor_reduce(out=mrow[:], in_=lm[:], axis=AX.X, op=OP.max)
            nmrow = sm(f"nmrow{s}")
            nc.vector.tensor_scalar_mul(out=nmrow[:], in0=mrow[:], scalar1=-1.0)
            expin = lpool.tile([PBLK, NS], BF16, tag=f"expin{s}", name=f"expin{s}")
            nc.vector.tensor_scalar(
                out=expin[:], in0=lm[:], scalar1=nmrow[:],
                scalar2=EXP_CLAMP, op0=OP.add, op1=OP.max,
            )
            expout = lpool.tile([PBLK, NS], BF16, tag=f"expout{s}", name=f"expout{s}")
            sumexp = sm(f"sumexp{s}")
            nc.scalar.activation(
                out=expout[:], in_=expin[:], func=AF.Exp, accum_out=sumexp[:],
            )
            lse = sm(f"lse{s}")
            nc.scalar.activation(out=lse[:], in_=sumexp[:], func=AF.Ln)
            nc.vector.tensor_add(out=lse[:], in0=lse[:], in1=mrow[:])

            def ce(lp, tag):
                mm = sm("mm" + tag)
                nc.vector.tensor_max(out=mm[:], in0=lp[:], in1=lse[:])
                nmm = sm("nmm" + tag)
                nc.vector.tensor_scalar_mul(out=nmm[:], in0=mm[:], scalar1=-1.0)
                e1 = sm("e1" + tag)
                nc.vector.tensor_scalar(
                    out=e1[:], in0=lp[:], scalar1=nmm[:], scalar2=EXP_CLAMP,
                    op0=OP.add, op1=OP.max,
                )
                nc.scalar.activation(out=e1[:], in_=e1[:], func=AF.Exp)
                e2 = sm("e2" + tag)
                nc.vector.tensor_scalar(
                    out=e2[:], in0=lse[:], scalar1=nmm[:], scalar2=EXP_CLAMP,
                    op0=OP.add, op1=OP.max,
                )
                nc.scalar.activation(out=e2[:], in_=e2[:], func=AF.Exp)
                s12 = sm("s12" + tag)
                nc.vector.tensor_add(out=s12[:], in0=e1[:], in1=e2[:])
                nc.scalar.activation(out=s12[:], in_=s12[:], func=AF.Ln)
                cev = sm("ce" + tag)
                nc.vector.tensor_add(out=cev[:], in0=s12[:], in1=mm[:])
                nc.vector.tensor_sub(out=cev[:], in0=cev[:], in1=lp[:])
                return cev

            cep = ce(lpos, f"p{s}")
            cenb = ce(lposnb, f"n{s}")

            # BML term (fnd is unscaled q.fn)
            maskf = sm(f"maskf{s}", BF16, w=M_MAX)
            nc.vector.tensor_scalar(
                out=maskf[:], in0=iota_f[:, :M_MAX], scalar1=fc_f[:],
                scalar2=None, op0=OP.is_lt,
            )
            fmask = sm(f"fmask{s}", BF16, w=M_MAX)
            nc.vector.tensor_mul(out=fmask[:], in0=fnd[s][:], in1=maskf[:])
            sfn = sm(f"sfn{s}")
            nc.vector.tensor_reduce(out=sfn[:], in_=fmask[:], axis=AX.X, op=OP.add)
            den = sm(f"den{s}")
            nc.vector.tensor_scalar_max(out=den[:], in0=fc_f[:], scalar1=1.0)
            rden = sm(f"rden{s}")
            nc.vector.reciprocal(out=rden[:], in_=den[:])
            simfn = sm(f"simfn{s}")
            nc.vector.tensor_mul(out=simfn[:], in0=sfn[:], in1=rden[:])
            delta = sm(f"delta{s}")
            nc.vector.tensor_sub(out=delta[:], in0=simfn[:], in1=simpos[:])
            r1 = sm(f"r1{s}")
            nc.scalar.activation(out=r1[:], in_=delta[:], func=AF.Relu,
                                 bias=alpha_t[:], scale=1.0)
            r2 = sm(f"r2{s}")
            nc.scalar.activation(out=r2[:], in_=delta[:], func=AF.Relu,
                                 bias=nbeta_t[:], scale=-1.0)
            bml = sm(f"bml{s}")
            nc.vector.tensor_add(out=bml[:], in0=r1[:], in1=r2[:])

            vh = sm(f"vh{s}")
            nc.vector.tensor_scalar(out=vh[:], in0=hc_f[:], scalar1=0.0,
                                    scalar2=None, op0=OP.is_gt)
            vf = sm(f"vf{s}")
            nc.vector.tensor_scalar(out=vf[:], in0=fc_f[:], scalar1=0.0,
                                    scalar2=None, op0=OP.is_gt)
            vb = sm(f"vb{s}")
            nc.vector.tensor_mul(out=vb[:], in0=vh[:], in1=vf[:])

            contrib = smpool.tile([PBLK, 5], F32, tag=f"contrib{s}",
                                  name=f"contrib{s}")
            nc.vector.tensor_mul(out=contrib[:, 0:1], in0=cep[:], in1=vh[:])
            nc.vector.tensor_mul(out=contrib[:, 1:2], in0=cenb[:], in1=vh[:])
            nc.vector.tensor_mul(out=contrib[:, 2:3], in0=bml[:], in1=vb[:])
            nc.vector.tensor_copy(out=contrib[:, 3:4], in_=vh[:])
            nc.vector.tensor_copy(out=contrib[:, 4:5], in_=vb[:])
            blk_contribs.append(contrib)

        tot = blk_contribs[0]
        nc.vector.tensor_add(out=tot[:], in0=tot[:], in1=blk_contribs[1][:])

        ps5 = rpool.tile([5, 1], F32, tag="ps5", name="ps5")
        nc.tensor.matmul(ps5[:], lhsT=tot[:], rhs=ones[:], start=True, stop=True)
        res = smpool.tile([5, 1], F32, tag="res", name="res")
        nc.scalar.copy(out=res[:], in_=ps5[:])
        nc.sync.dma_start(out=out[:], in_=res[:])


def _stream_sizes(bounds):
    ht_w = 0
    scr_w = 0
    for s in range(NSLOT):
        for nact in _rounds(bounds[s]):
            ht_w += KT * nact * COLS
            scr_w += COLS
        ht_w += (M_MAX // JC) * KT * NB * COLS
        scr_w += (M_MAX // JC) * COLS
    return ht_w, scr_w


def _build(bounds):
    ht_w, scr_w = _stream_sizes(bounds)
    nc = bacc.Bacc("TRN2", target_bir_lowering=False, debug=False)
    ht = nc.dram_tensor("ht", [PBLK, ht_w], F8, kind="ExternalInput")
    qt = nc.dram_tensor("qt", [NSLOT, PBLK, KT, PBLK], F8, kind="ExternalInput")
    q = nc.dram_tensor("q", [NSLOT, PBLK, D], F32, kind="ExternalInput")
    k = nc.dram_tensor("k", [NSLOT, PBLK, D], F32, kind="ExternalInput")
    k2 = nc.dram_tensor("k2", [NSLOT, PBLK, D], F32, kind="ExternalInput")
    hc = nc.dram_tensor("hn_counts", [NSLOT, PBLK, 1], I32, kind="ExternalInput")
    fc = nc.dram_tensor("fn_counts", [NSLOT, PBLK, 1], I32, kind="ExternalInput")
    scr = nc.dram_tensor("scr", [PBLK, scr_w], BF16, kind="Internal")
    out = nc.dram_tensor("out", [5, 1], F32, kind="ExternalOutput")
    with tile.TileContext(nc) as tc:
        _emit(tc, bounds, ht, qt, scr, q, k, k2, hc, fc, out)
    nc.compile()
    return nc


_NC_CACHE = {}


def _get_nc(bounds):
    if bounds not in _NC_CACHE:
        _NC_CACHE[bounds] = _build(bounds)
    return _NC_CACHE[bounds]


def _r16(x):
    return int(min(max(JC, -(-int(x) // JC) * JC), N_MAX))


def plan(hn_counts):
    """Sorted block schedule: core c, slot s, lane l <- rank s*1024+l*8+c."""
    hn_counts = np.asarray(hn_counts)
    order = np.argsort(-hn_counts, kind="stable")
    cs = hn_counts[order]
    ids = order.reshape(NSLOT, PBLK, N_CORES)     # [slot][lane][core]
    bounds = tuple(
        tuple(_r16(cs[s * (PBLK * N_CORES) + G * N_CORES * b]) for b in range(NB))
        for s in range(NSLOT)
    )
    return ids, bounds


def _pack_T(x8, ids_cs, n0, n1):
    """(lanes, n, d) fp8 -> (p, kt, g-major cols) for rounds [n0, n1)."""
    a = x8[ids_cs, n0:n1, :]                      # (nl, JC, D)
    nl = len(ids_cs)
    a = a.reshape(nl, n1 - n0, KT, PBLK)          # (lane, j, kt, p)
    return a.transpose(3, 2, 0, 1)                # (p, kt, lane, j)


def make_in_maps(q, k, k2, hn, fn, hn_counts, fn_counts):
    import ml_dtypes
    E4 = ml_dtypes.float8_e4m3
    q = np.asarray(q, np.float32)
    k = np.asarray(k, np.float32)
    k2 = np.asarray(k2, np.float32)
    hn = np.asarray(hn, np.float32)
    fn = np.asarray(fn, np.float32)
    hn_counts = np.asarray(hn_counts, np.int32)
    fn_counts = np.asarray(fn_counts, np.int32)

    ids, bounds = plan(hn_counts)
    q8 = q.astype(E4)
    hn8 = hn.astype(E4)
    fn8 = fn.astype(E4)
    ht_w, scr_w = _stream_sizes(bounds)

    in_maps = []
    for c in range(N_CORES):
        parts = []
        qt = np.empty((NSLOT, PBLK, KT, PBLK), E4)
        for s in range(NSLOT):
            ids_cs = ids[s, :, c]                 # (128,) sample ids, sorted
            qt[s] = (q8[ids_cs].reshape(PBLK, KT, PBLK)   # (lane, kt, p)
                     .transpose(2, 1, 0))                  # (p, kt, lane)
            nr = bounds[s][0] // JC
            nacts = _rounds(bounds[s])
            for r in range(nr):
                nact = nacts[r]
                a = _pack_T(hn8, ids_cs[:G * nact], JC * r, JC * (r + 1))
                parts.append(np.ascontiguousarray(a).reshape(PBLK, -1))
            for r in range(M_MAX // JC):
                a = _pack_T(fn8, ids_cs, JC * r, JC * (r + 1))
                parts.append(np.ascontiguousarray(a).reshape(PBLK, -1))
        ht = np.concatenate(parts, axis=1)
        assert ht.shape == (PBLK, ht_w), (ht.shape, ht_w)
        lane_ids = ids[:, :, c]                   # (NSLOT, PBLK)
        in_maps.append({
            "ht": ht,
            "qt": qt,
            "q": q[lane_ids],
            "k": k[lane_ids],
            "k2": k2[lane_ids],
            "hn_counts": hn_counts[lane_ids][..., None],
            "fn_counts": fn_counts[lane_ids][..., None],
        })
    return in_maps, bounds


def combine_partials(results):
    parts = np.stack([np.asarray(r["out"], np.float64).reshape(5) for r in results])
    cl_s, clnb_s, bml_s, nv, nb = parts.sum(axis=0)
    n_valid = max(nv, 1.0)
    cl = cl_s / n_valid
    clnb = clnb_s / n_valid
    bml_mean = (bml_s / nb) if nb > 0 else 0.0
    lbml = LAMBDA_BML * bml_mean
    tot = cl + clnb + lbml
    return np.array([tot, cl, lbml, clnb], np.float32)


def run_spmd(in_maps, bounds, **kwargs):
    nc = _get_nc(bounds)
    return bass_utils.run_bass_kernel_spmd(
        nc, in_maps, core_ids=list(range(N_CORES)), **kwargs
    )


def kernel(q, k, k2, hn, fn, hn_counts, fn_counts):
    in_maps, bounds = make_in_maps(q, k, k2, hn, fn, hn_counts, fn_counts)
    res = run_spmd(in_maps, bounds)
    return combine_partials(res.results)
